# revision 83
# baseline (speedup 1.0000x reference)
"""Multi-head attention (B=4, S=2048, D=1024, H=16, dk=dv=64) on 8 TRN2 cores.

Sharding: core c = 2*b + hg handles batch b = c//2 and heads
[hg*8, hg*8+8). Each core computes a partial output
(its 8 heads' contribution through Wo); the host adds the two partials
per batch.

Per-core device pipeline (matmul inputs bf16, PSUM accumulation fp32).
The kernel is issue-ordered so the list scheduler keeps the PE gap-free
(p-state at max) and the ACT exp stream (the second-busiest engine)
starts ~14us in and never starves:

  - k-proj block 0 and q-proj(qb0,p0) are issued first; attention
    (qb0,p0) scores begin immediately after.
  - all remaining projection work (k blocks 1-3, q pairs, v chunks) is
    issued as PE filler interleaved into the attention g-loops of qb0,
    so the PE always has ready work while exp(g) -> mix(g) dependencies
    drain. v chunk t is projected just-in-time before mix needs it.
  - for qb>0, the fillers are the previous qb's Wo matmuls and the next
    qb's q projection.
  - scores^T per head pair are K=64 matmuls on partition halves
    (h0: partitions 0:64 -> PE tile (0,0); h1: 64:128 -> tile (64,0)),
    [128 keys, 512 q] fp32 in PSUM, two key chunks per [128,1024] PSUM
    tile so each ScalarE exp instruction covers 2 banks.
  - mix^T + softmax sums in one matmul: lhsT = vh_aug [128 keys, 65]
    (col 64 = mask), rhs = exp chunk half; h0/h1 accumulate into the
    two banks of one [128,1024] PSUM tile over the 16 key chunks.
  - mix for group g is issued one g-iteration behind its exp so the PE
    never sits adjacent to the ACT dependency (weights prefetch).
  - normalize (PE-free): evacuate mix rows 0:65 to SBUF (frees the
    PSUM tile after one DVE op), bounce the sums row (partition 64)
    through a DRAM scratch tile to broadcast it over 64 partitions,
    reciprocal on DVE, scale multiplies on the idle GpSimd. The final
    pair instead uses a K=1 f32r PE-broadcast matmul + DVE multiplies
    straight out of PSUM (lowest latency - it is the kernel tail).
    h1's normalized tile is DMA-shifted to partitions 64-127 so each
    pair's mix^T is one [128, 512] tile (e on partitions).
  - out += mixT_norm.T @ Wo: dense K=128 bf16 matmuls accumulating
    over the 4 pairs; DVE evac fp32 -> DMA to HBM. The last qb's Wo
    runs p-major across 6 concurrent PSUM accumulators so only the
    final pair's 6 matmuls + evacs trail the last normalize.

PSUM: sc ring 2x[128,1024] (scores + qb0 projection accumulators +
the tail broadcast) = 4 banks, mix 1x[128,1024] = 2 banks, aux ring
2x[128,512] (projection pj + Wo accumulators) = 2 banks.
"""

import numpy as np

B, S, D = 4, 2048, 1024
H, DK, DV = 16, 64, 64
HC = 8          # heads per core
NP = HC // 2    # head pairs per core
NCORES = 8
NC_CHUNKS = D // 128    # 8 contraction chunks over D
NKC = S // 128          # 16 key chunks
NQB = S // 512          # 4 query blocks
VW = HC * (DV + 1)      # vh storage: 65 cols per head (dv | mask)

_COMPILED = {}


def _build_nc():
    import concourse.tile as tile
    from concourse import bacc, mybir
    from contextlib import ExitStack

    F32 = mybir.dt.float32
    F32R = mybir.dt.float32r
    BF16 = mybir.dt.bfloat16
    EXP = mybir.ActivationFunctionType.Exp
    COPY = mybir.ActivationFunctionType.Copy

    nc = bacc.Bacc("TRN2", target_bir_lowering=False, debug=False,
                   num_devices=NCORES)

    qT = nc.dram_tensor("qT", [D, S], BF16, kind="ExternalInput").ap()
    kT = nc.dram_tensor("kT", [D, S], BF16, kind="ExternalInput").ap()
    vT = nc.dram_tensor("vT", [D, S], BF16, kind="ExternalInput").ap()
    wq = nc.dram_tensor("wq", [D, HC * DK], BF16, kind="ExternalInput").ap()
    wk = nc.dram_tensor("wk", [D, HC * DK], BF16, kind="ExternalInput").ap()
    wv = nc.dram_tensor("wv", [D, HC * DV], BF16, kind="ExternalInput").ap()
    wo = nc.dram_tensor("wo", [HC * DV, D], BF16, kind="ExternalInput").ap()
    maskr = nc.dram_tensor("maskr", [128, NKC], F32, kind="ExternalInput").ap()
    out = nc.dram_tensor("out", [S, D], F32, kind="ExternalOutput").ap()

    with tile.TileContext(nc) as tc:
        with ExitStack() as ctx:
            const_pool = ctx.enter_context(tc.tile_pool(name="const", bufs=1))
            w_pool = ctx.enter_context(tc.tile_pool(name="weights", bufs=1))
            act_pool = ctx.enter_context(tc.tile_pool(name="acts", bufs=1))
            # 5 blocks of 8 stage tiles live at once (k0-k3 + q0); later
            # q blocks wrap onto long-drained slots
            st_pool = ctx.enter_context(
                tc.tile_pool(name="stage", bufs=5 * NC_CHUNKS))
            vt_pool = ctx.enter_context(tc.tile_pool(name="vstage", bufs=2))
            exp_pool = ctx.enter_context(tc.tile_pool(name="exp", bufs=6))
            norm_pool = ctx.enter_context(tc.tile_pool(name="norm", bufs=2 * NP))
            rec_pool = ctx.enter_context(tc.tile_pool(name="rec", bufs=4))
            sums_pool = ctx.enter_context(tc.tile_pool(name="sums", bufs=2))
            sh_pool = ctx.enter_context(tc.tile_pool(name="sh", bufs=4))
            osb_pool = ctx.enter_context(tc.tile_pool(name="outsb", bufs=4))
            sc_pool = ctx.enter_context(
                tc.tile_pool(name="scpsum", bufs=2, space="PSUM"))
            mix_pool = ctx.enter_context(
                tc.tile_pool(name="mxpsum", bufs=1, space="PSUM"))
            aux_pool = ctx.enter_context(
                tc.tile_pool(name="auxpsum", bufs=2, space="PSUM"))
            dram_pool = ctx.enter_context(
                tc.tile_pool(name="dscratch", bufs=4, space="DRAM"))

            mask_sb = const_pool.tile([128, NKC], F32)
            nc.sync.dma_start(mask_sb[:], maskr[:])
            # PE warmup: the PE would otherwise idle ~13us waiting for
            # the first weight/staging DMAs, then pay the 0.65/1.2GHz
            # p-state ramp on the real projections. Dummy matmuls on
            # resident SBUF data bridge the wait at full ramp.
            warm_sb = const_pool.tile([128, 512], BF16)
            nc.vector.memset(warm_sb[:], 0.0)
            warm_ps = aux_pool.tile([128, 512], F32, tag="aux",
                                    name="warmps")
            for i in range(48):
                nc.tensor.matmul(warm_ps[:], lhsT=warm_sb[:, 0:128],
                                 rhs=warm_sb[:],
                                 start=(i == 0), stop=(i == 47))
            warm_out = const_pool.tile([128, 512], F32)
            nc.vector.tensor_copy(warm_out[:], warm_ps[:])
            ones_sb = const_pool.tile([128, HC], BF16)
            nc.vector.memset(ones_sb[:], 1.0)
            # K=1 f32r ones row for the tail's PE sum-broadcast
            # (memset cannot write f32r; bounce through an f32 tile)
            e_ones32 = const_pool.tile([65, 64], F32)
            nc.vector.memset(e_ones32[:], 1.0)
            e_ones = const_pool.tile([65, 64], F32R)
            nc.vector.tensor_copy(e_ones[:], e_ones32[:])

            # DMA priority order: the head's critical path is
            # wk + k staging (first projections), then wq + q staging;
            # wv is needed at attention start and wo only at qb1.
            wq_sb = w_pool.tile([128, NC_CHUNKS * 512], BF16, tag="wq")
            wk_sb = w_pool.tile([128, NC_CHUNKS * 512], BF16, tag="wk")
            wv_sb = w_pool.tile([128, NC_CHUNKS * 512], BF16, tag="wv")
            wo_sb = w_pool.tile([128, NP * 1024], BF16, tag="wo")
            # head critical path: wk then kT-block0, whole chunks (1KB
            # partition lines - smaller splits lose DMA efficiency)
            for c in range(NC_CHUNKS):
                nc.sync.dma_start(wk_sb[:, c * 512:(c + 1) * 512],
                                  wk[c * 128:(c + 1) * 128, :])

            # persistent activations. khT/vh are single tiles (written
            # in slices, region-tracked) so consecutive attention
            # matmuls switch lhsT within one tile - cheaper LDWEIGHTS.
            qhTb = [[act_pool.tile([128, 512], BF16, tag=f"qhT{p}_{b}",
                                   name=f"qhT{p}_{b}") for b in range(NQB)]
                    for p in range(NP)]
            khT = [act_pool.tile([128, S], BF16, tag=f"khT{p}",
                                 name=f"khT{p}") for p in range(NP)]
            khTb = [[khT[p][:, b * 512:(b + 1) * 512] for b in range(NQB)]
                    for p in range(NP)]
            vhs_all = act_pool.tile([128, NKC * VW], BF16, tag="vhall")
            vhs = [vhs_all[:, t * VW:(t + 1) * VW] for t in range(NKC)]

            def stage_block(src, blk, split=False):
                stg = []
                for c in range(NC_CHUNKS):
                    t = st_pool.tile([128, 512], BF16, tag="stage",
                                     name=f"stg{c}")
                    nsp = (4 if c < 2 else 2) if split else 1
                    w = 512 // nsp
                    for h in range(nsp):
                        nc.sync.dma_start(
                            t[:, h * w:(h + 1) * w],
                            src[c * 128:(c + 1) * 128,
                                blk * 512 + h * w:blk * 512 + (h + 1) * w])
                    stg.append(t)
                return stg

            def proj_pair(stg, wsb, dst_tile, p, pool=None, act_evac=False):
                if pool is None:
                    ps = aux_pool.tile([128, 512], F32, tag="aux")
                else:
                    # borrow half an sc-ring tile (qb0 fillers: the sc
                    # ring has slack while attention is PE-bound)
                    pst = pool.tile([128, 1024], F32, tag="sc", name="pjsc")
                    ps = pst[:, 0:512]
                for c in range(NC_CHUNKS):
                    nc.tensor.matmul(
                        ps[:],
                        lhsT=wsb[:, c * 512 + p * 128:
                                 c * 512 + (p + 1) * 128],
                        rhs=stg[c][:],
                        start=(c == 0), stop=(c == NC_CHUNKS - 1))
                if act_evac:
                    # qb0 era: ScalarE is idle; keep DVE off the critical
                    # path of PSUM slot recycling
                    nc.scalar.activation(dst_tile[:], ps[:], COPY)
                else:
                    nc.vector.tensor_copy(dst_tile[:], ps[:])

            # vT is staged in 512-token quarters (large 1KB-line DMAs,
            # double buffered) so v-proj matmuls never wait on staging
            vt_cur = {}

            def stage_vq(vq):
                vt = vt_pool.tile([128, NC_CHUNKS * 512], BF16, tag="vt",
                                  name=f"vq{vq}")
                for c in range(NC_CHUNKS):
                    nc.sync.dma_start(
                        vt[:, c * 512:(c + 1) * 512],
                        vT[c * 128:(c + 1) * 128,
                           vq * 512:(vq + 1) * 512])
                vt_cur[vq] = vt

            def vproj_chunk(t):
                vt = vt_cur[t // 4]
                o = t % 4
                ps = aux_pool.tile([128, 512], F32, tag="aux")
                for c in range(NC_CHUNKS):
                    nc.tensor.matmul(
                        ps[:],
                        lhsT=vt[:, c * 512 + o * 128:
                                c * 512 + (o + 1) * 128],
                        rhs=wv_sb[:, c * 512:(c + 1) * 512],
                        start=(c == 0), stop=(c == NC_CHUNKS - 1))
                dst_dv = vhs[t][:, 0:VW].rearrange(
                    "p (h x) -> p h x", x=DV + 1)[:, :, 0:DV]
                src_dv = ps[:].rearrange("p (h x) -> p h x", x=DV)
                nc.scalar.activation(dst_dv, src_dv, COPY,
                                     scale=mask_sb[:, t:t + 1])
                dst_m = vhs[t][:, 0:VW].rearrange(
                    "p (h x) -> p h x", x=DV + 1)[:, :, DV:DV + 1]
                src_m = ones_sb[:, 0:HC].rearrange("p (h x) -> p h x", x=1)
                nc.vector.tensor_scalar_mul(dst_m, src_m,
                                            mask_sb[:, t:t + 1])

            # Wo for one (qb, tt, dh) triple, issued one matmul at a time
            # (4 accumulating steps + evac) so it spreads as PE filler
            wo_state = {}

            def wo_single(qb, normT, tt, dh, p):
                if p == 0:
                    wo_state[(tt, dh)] = aux_pool.tile(
                        [128, 512], F32, tag="aux", name=f"wps{tt}{dh}")
                wps = wo_state[(tt, dh)]
                nc.tensor.matmul(
                    wps[:],
                    lhsT=normT[p][:, tt * 128:(tt + 1) * 128],
                    rhs=wo_sb[:, p * 1024 + dh * 512:
                              p * 1024 + (dh + 1) * 512],
                    start=(p == 0), stop=(p == NP - 1))
                if p == NP - 1:
                    osb = osb_pool.tile([128, 512], F32, tag="osb")
                    nc.vector.tensor_copy(osb[:], wps[:])
                    nc.sync.dma_start(
                        out[qb * 512 + tt * 128:qb * 512 + (tt + 1) * 128,
                            dh * 512:(dh + 1) * 512], osb[:])

            def wo_piece(qb, normT, tt, dh):
                for p in range(NP):
                    wo_single(qb, normT, tt, dh, p)

            # ---- stage + project k block 0 / q block 0 for pair 0 ----
            kstg = [None] * NQB
            qstg = [None] * NQB
            kstg[0] = stage_block(kT, 0)
            for c in range(NC_CHUNKS):
                nc.sync.dma_start(wq_sb[:, c * 512:(c + 1) * 512],
                                  wq[c * 128:(c + 1) * 128, :])
            qstg[0] = stage_block(qT, 0)
            for c in range(NC_CHUNKS):
                nc.sync.dma_start(wv_sb[:, c * 512:(c + 1) * 512],
                                  wv[c * 128:(c + 1) * 128, :])
            proj_pair(kstg[0], wk_sb, khTb[0][0], 0, act_evac=True)
            proj_pair(qstg[0], wq_sb, qhTb[0][0], 0, act_evac=True)
            stage_vq(0)
            stage_vq(1)
            for p in range(NP):
                nc.sync.dma_start(wo_sb[:, p * 1024:(p + 1) * 1024],
                                  wo[p * 128:(p + 1) * 128, :])

            # filler thunks, one consumed at the top of each attention
            # g-iteration. The order guarantees every tile's write is
            # issued before its first read (pair p's blocks land during
            # pair p-1's loop, with p0's later k blocks interleaved
            # just ahead of the scores that need them).
            def kf(kb, p):
                return lambda: proj_pair(kstg[kb], wk_sb, khTb[p][kb], p,
                                         pool=sc_pool, act_evac=True)

            def qf(qb, p, pool=None):
                return lambda: proj_pair(qstg[qb], wq_sb, qhTb[p][qb], p,
                                         pool=pool,
                                         act_evac=pool is not None)

            # q-projection issued one matmul at a time (steady-state
            # filler granularity; the aux tile is held across 8 slots)
            pj_state = {}

            def qsingle(qb, p, c):
                def f():
                    if c == 0:
                        pj_state[(qb, p)] = aux_pool.tile(
                            [128, 512], F32, tag="aux",
                            name=f"qpj{qb}_{p}")
                    ps = pj_state[(qb, p)]
                    nc.tensor.matmul(
                        ps[:],
                        lhsT=wq_sb[:, c * 512 + p * 128:
                                   c * 512 + (p + 1) * 128],
                        rhs=qstg[qb][c][:],
                        start=(c == 0), stop=(c == NC_CHUNKS - 1))
                    if c == NC_CHUNKS - 1:
                        nc.vector.tensor_copy(qhTb[p][qb][:], ps[:])
                return f

            def qb0_fillers():
                for kb in range(1, NQB):
                    kstg[kb] = stage_block(kT, kb)
                # None: no filler in the very first g iteration - lets
                # the attention pipeline warm up before the sc ring is
                # borrowed for projection accumulators.
                fills = [None, kf(1, 0), kf(0, 1), kf(2, 0), kf(0, 2),
                         kf(3, 0), kf(0, 3), qf(0, 1, sc_pool), kf(1, 1),
                         kf(2, 1), kf(3, 1), qf(0, 2, sc_pool), kf(1, 2),
                         kf(2, 2), kf(3, 2), qf(0, 3, sc_pool), kf(1, 3),
                         kf(2, 3), kf(3, 3)]

                def stage_q1():
                    qstg[1] = stage_block(qT, 1)
                fills.append(stage_q1)
                for p in range(NP):
                    fills.append(qf(1, p, sc_pool))
                return fills

            # completed blocks' Wo matmuls wait in a FIFO consumed across
            # later blocks' filler slots: qb1/qb2 are PE-bound (fillers
            # cost wall-clock 1:1) while qb3 has ACT-bound slack where
            # deferred Wo work is nearly free. Caps are multiples of 4 so
            # a Wo accumulation group never splits across blocks.
            wo_queue = []
            WO_CAP = {1: 24, 2: 28, 3: 10 ** 6}

            def qbn_fillers(qb):
                fills = []
                has_q = qb + 1 < NQB
                if has_q:
                    def stage_qn():
                        qstg[qb + 1] = stage_block(qT, qb + 1)
                    fills.append(stage_qn)
                nwo = min(WO_CAP[qb], len(wo_queue))
                wos = [wo_queue.pop(0) for _ in range(nwo)]
                qi = 0
                # interleave: one 4-matmul Wo group, then one 8-matmul
                # q-projection group (the two aux slots alternate)
                while wos or (has_q and qi < NP):
                    fills += wos[:4]
                    del wos[:4]
                    if has_q and qi < NP:
                        for c in range(NC_CHUNKS):
                            fills.append(qsingle(qb + 1, qi, c))
                        qi += 1
                return fills

            prev_normT = None
            for qb in range(NQB):
                fills = (qb0_fillers() if qb == 0
                         else qbn_fillers(qb))
                fills = iter(fills)

                def filler():
                    f = next(fills, None)
                    if f is not None and callable(f):
                        f()

                normT = []
                for p in range(NP):
                    h0, h1 = 2 * p, 2 * p + 1
                    mix2 = mix_pool.tile([128, 1024], F32, tag="mix")
                    mixP = mix2[:, 0:512]
                    mixR = mix2[:, 512:1024]
                    l0 = slice(h0 * 65, h0 * 65 + 65)
                    l1 = slice(h1 * 65, h1 * 65 + 65)

                    def mix_g(g, exs):
                        ex0, ex1 = exs
                        for s2 in range(2):
                            kc = 2 * g + s2
                            esl = slice(s2 * 512, (s2 + 1) * 512)
                            nc.tensor.matmul(
                                mixP[0:65, :],
                                lhsT=vhs[kc][:, l0], rhs=ex0[:, esl],
                                start=(kc == 0), stop=(kc == NKC - 1))
                        for s2 in range(2):
                            kc = 2 * g + s2
                            esl = slice(s2 * 512, (s2 + 1) * 512)
                            nc.tensor.matmul(
                                mixR[0:65, :],
                                lhsT=vhs[kc][:, l1], rhs=ex1[:, esl],
                                start=(kc == 0), stop=(kc == NKC - 1))

                    # mix for group g is issued one iteration behind its
                    # exp, so the PE never sits adjacent to the ACT
                    # dependency (weights prefetch, no stall).
                    pend = None
                    for g in range(NKC // 2):
                        filler()
                        if not (qb == 0 and p == 0):
                            filler()
                        sc0 = sc_pool.tile([128, 1024], F32, tag="sc")
                        sc1 = sc_pool.tile([128, 1024], F32, tag="sc")
                        for s2 in range(2):
                            kc = 2 * g + s2
                            kb, ko = kc // 4, kc % 4
                            ksl = slice(ko * 128, (ko + 1) * 128)
                            nc.tensor.matmul(
                                sc0[:, s2 * 512:(s2 + 1) * 512],
                                lhsT=khTb[p][kb][0:64, ksl],
                                rhs=qhTb[p][qb][0:64, :],
                                start=True, stop=True)
                            nc.tensor.matmul(
                                sc1[:, s2 * 512:(s2 + 1) * 512],
                                lhsT=khTb[p][kb][64:128, ksl],
                                rhs=qhTb[p][qb][64:128, :],
                                start=True, stop=True)
                        if qb == 0 and p == 0:
                            # project v chunks just-in-time for mix;
                            # prefetch the next vT quarter (ring of 2)
                            if g == 2:
                                stage_vq(2)
                            elif g == 4:
                                stage_vq(3)
                            vproj_chunk(2 * g)
                            vproj_chunk(2 * g + 1)
                        ex0 = exp_pool.tile([128, 1024], BF16, tag="exp")
                        ex1 = exp_pool.tile([128, 1024], BF16, tag="exp")
                        nc.scalar.activation(ex0[:], sc0[:], EXP)
                        nc.scalar.activation(ex1[:], sc1[:], EXP)
                        if pend is not None:
                            mix_g(g - 1, pend)
                        pend = (ex0, ex1)
                    mix_g(NKC // 2 - 1, pend)
                    nt = norm_pool.tile([128, 512], BF16, tag="norm")
                    normT.append(nt)
                    if qb == NQB - 1 and p == NP - 1:
                        # kernel tail: lowest-latency normalize. Broadcast
                        # the sums row with a K=1 f32r matmul (PE is idle
                        # here), multiply straight out of PSUM on DVE.
                        su_r = sums_pool.tile([65, 1024], F32R, tag="sumr",
                                              name="sur")
                        nc.vector.tensor_copy(su_r[64:65, :], mix2[64:65, :])
                        bcp = sc_pool.tile([128, 1024], F32, tag="sc",
                                           name="bcp")
                        nc.tensor.matmul(
                            bcp[0:64, 0:512], lhsT=e_ones[64:65, :],
                            rhs=su_r[64:65, 0:512], start=True, stop=True)
                        nc.tensor.matmul(
                            bcp[0:64, 512:1024], lhsT=e_ones[64:65, :],
                            rhs=su_r[64:65, 512:1024], start=True, stop=True)
                        recb = rec_pool.tile([64, 1024], F32, tag="rec")
                        nc.vector.reciprocal_approx_fast(recb[:],
                                                         bcp[0:64, :])
                        nc.vector.tensor_mul(nt[0:64, :], mix2[0:64, 0:512],
                                             recb[:, 0:512])
                        sh1 = sh_pool.tile([64, 512], BF16, tag="sh1")
                        nc.vector.tensor_mul(sh1[:], mix2[0:64, 512:1024],
                                             recb[:, 512:1024])
                        nc.sync.dma_start(nt[64:128, :], sh1[:])
                    else:
                        # normalize (no PE involvement): evacuate mix rows
                        # 0:65 to SBUF (frees the PSUM tile), bounce the
                        # sums row through DRAM to broadcast it over
                        # partitions, reciprocal on DVE, scale on GpSimd.
                        madd = sums_pool.tile([65, 1024], F32, tag="sums")
                        nc.vector.tensor_copy(madd[:], mix2[0:65, :])
                        dsc = dram_pool.tile([1, 1024], F32, tag="dsc")
                        nc.sync.dma_start(dsc[:], madd[64:65, :])
                        rin = rec_pool.tile([64, 1024], F32, tag="rec")
                        nc.sync.dma_start(
                            rin[:], dsc[0:1, :].to_broadcast((64, 1024)))
                        recb = rec_pool.tile([64, 1024], F32, tag="rec")
                        nc.vector.reciprocal_approx_fast(recb[:], rin[:])
                        nc.gpsimd.tensor_mul(nt[0:64, :], madd[0:64, 0:512],
                                             recb[:, 0:512])
                        sh1 = sh_pool.tile([64, 512], BF16, tag="sh1")
                        nc.gpsimd.tensor_mul(sh1[:], madd[0:64, 512:1024],
                                             recb[:, 512:1024])
                        nc.sync.dma_start(nt[64:128, :], sh1[:])

                # drain any unissued fillers for this qb
                for f in fills:
                    if callable(f):
                        f()
                prev_normT = normT
                if qb < NQB - 1:
                    for tt in range(4):
                        for dh in range(2):
                            for p in range(NP):
                                wo_queue.append(
                                    lambda qb=qb, normT=normT, tt=tt,
                                    dh=dh, p=p: wo_single(
                                        qb, normT, tt, dh, p))

            # final block's Wo: 6 groups held concurrently across the
            # aux + sc PSUM banks, issued p-major so the p0-p2 matmuls
            # execute during the last pair's (ACT-bound) attention; only
            # the p3 matmuls + evacs trail the last normalize.
            combos = [(tt, dh) for tt in range(4) for dh in range(2)]
            wo_tiles = [
                aux_pool.tile([128, 512], F32, tag="aux", name="fwa0"),
                aux_pool.tile([128, 512], F32, tag="aux", name="fwa1"),
            ]
            fwsc0 = sc_pool.tile([128, 1024], F32, tag="sc", name="fwsc0")
            fwsc1 = sc_pool.tile([128, 1024], F32, tag="sc", name="fwsc1")
            wo_tiles += [fwsc0[:, 0:512], fwsc0[:, 512:1024],
                         fwsc1[:, 0:512], fwsc1[:, 512:1024]]
            # the mix banks free once the tail normalize multiplies have
            # read them - the last two groups ride there, so all 8 run
            # concurrently and only p3 matmuls + evacs trail the norm
            fwmx = mix_pool.tile([128, 1024], F32, tag="mix", name="fwmx")
            wo_tiles += [fwmx[:, 0:512], fwmx[:, 512:1024]]
            for p in range(NP):
                for i in range(8):
                    tt, dh = combos[i]
                    nc.tensor.matmul(
                        wo_tiles[i][:],
                        lhsT=prev_normT[p][:, tt * 128:(tt + 1) * 128],
                        rhs=wo_sb[:, p * 1024 + dh * 512:
                                  p * 1024 + (dh + 1) * 512],
                        start=(p == 0), stop=(p == NP - 1))
            for i in range(8):
                tt, dh = combos[i]
                osb = osb_pool.tile([128, 512], F32, tag="osb")
                nc.vector.tensor_copy(osb[:], wo_tiles[i][:])
                nc.sync.dma_start(
                    out[(NQB - 1) * 512 + tt * 128:
                        (NQB - 1) * 512 + (tt + 1) * 128,
                        dh * 512:(dh + 1) * 512], osb[:])

    nc.compile()
    return nc


def _get_nc():
    if "nc" not in _COMPILED:
        _COMPILED["nc"] = _build_nc()
    return _COMPILED["nc"]


def _shard_inputs(q, k, v, mask, Wq, Wk, Wv, Wo):
    """Build the per-core input maps (host-side layout prep)."""
    import ml_dtypes

    bf16 = ml_dtypes.bfloat16
    in_maps = []
    maskf = np.asarray(mask).astype(np.float32)
    q = np.asarray(q, np.float32)
    k = np.asarray(k, np.float32)
    v = np.asarray(v, np.float32)
    Wq = np.asarray(Wq, np.float32)
    Wk = np.asarray(Wk, np.float32)
    Wv = np.asarray(Wv, np.float32)
    Wo = np.asarray(Wo, np.float32)
    scale = np.float32(1.0 / np.sqrt(DK))
    for c in range(NCORES):
        b, hg = c // 2, c % 2
        hs = hg * HC
        m = {
            "qT": np.ascontiguousarray(q[b].T).astype(bf16),
            "kT": np.ascontiguousarray(k[b].T).astype(bf16),
            "vT": np.ascontiguousarray(v[b].T).astype(bf16),
            # head-major col blocks; fold 1/sqrt(dk) into Wq
            "wq": np.ascontiguousarray(
                Wq[hs:hs + HC].transpose(1, 0, 2).reshape(D, HC * DK) * scale
            ).astype(bf16),
            "wk": np.ascontiguousarray(
                Wk[hs:hs + HC].transpose(1, 0, 2).reshape(D, HC * DK)
            ).astype(bf16),
            "wv": np.ascontiguousarray(
                Wv[hs:hs + HC].transpose(1, 0, 2).reshape(D, HC * DV)
            ).astype(bf16),
            "wo": np.ascontiguousarray(Wo[hs * DV:(hs + HC) * DV]).astype(bf16),
            "maskr": np.ascontiguousarray(
                maskf[b].reshape(NKC, 128).T).astype(np.float32),
        }
        in_maps.append(m)
    return in_maps


def kernel(q, k, v, mask, Wq, Wk, Wv, Wo, _trace=False):
    from concourse.bass_utils import run_bass_kernel_spmd

    nc = _get_nc()
    in_maps = _shard_inputs(q, k, v, mask, Wq, Wk, Wv, Wo)
    res = run_bass_kernel_spmd(nc, in_maps, list(range(NCORES)),
                               trace=_trace)
    out = np.zeros((B, S, D), np.float32)
    for c in range(NCORES):
        out[c // 2] += res.results[c]["out"]
    if _trace:
        _COMPILED["last_result"] = res
    return out


# revision 84
# speedup vs baseline: 1.0007x; 1.0007x over previous
"""Multi-head attention (B=4, S=2048, D=1024, H=16, dk=dv=64) on 8 TRN2 cores.

Sharding: core c = 2*b + hg handles batch b = c//2 and heads
[hg*8, hg*8+8). Each core computes a partial output
(its 8 heads' contribution through Wo); the host adds the two partials
per batch.

Per-core device pipeline (matmul inputs bf16, PSUM accumulation fp32).
The kernel is issue-ordered so the list scheduler keeps the PE gap-free
(p-state at max) and the ACT exp stream (the second-busiest engine)
starts ~14us in and never starves:

  - k-proj block 0 and q-proj(qb0,p0) are issued first; attention
    (qb0,p0) scores begin immediately after.
  - all remaining projection work (k blocks 1-3, q pairs, v chunks) is
    issued as PE filler interleaved into the attention g-loops of qb0,
    so the PE always has ready work while exp(g) -> mix(g) dependencies
    drain. v chunk t is projected just-in-time before mix needs it.
  - for qb>0, the fillers are the previous qb's Wo matmuls and the next
    qb's q projection.
  - scores^T per head pair are K=64 matmuls on partition halves
    (h0: partitions 0:64 -> PE tile (0,0); h1: 64:128 -> tile (64,0)),
    [128 keys, 512 q] fp32 in PSUM, two key chunks per [128,1024] PSUM
    tile so each ScalarE exp instruction covers 2 banks.
  - mix^T + softmax sums in one matmul: lhsT = vh_aug [128 keys, 65]
    (col 64 = mask), rhs = exp chunk half; h0/h1 accumulate into the
    two banks of one [128,1024] PSUM tile over the 16 key chunks.
  - mix for group g is issued one g-iteration behind its exp so the PE
    never sits adjacent to the ACT dependency (weights prefetch).
  - normalize (PE-free): evacuate mix rows 0:65 to SBUF (frees the
    PSUM tile after one DVE op), bounce the sums row (partition 64)
    through a DRAM scratch tile to broadcast it over 64 partitions,
    reciprocal on DVE, scale multiplies on the idle GpSimd. The final
    pair instead uses a K=1 f32r PE-broadcast matmul + DVE multiplies
    straight out of PSUM (lowest latency - it is the kernel tail).
    h1's normalized tile is DMA-shifted to partitions 64-127 so each
    pair's mix^T is one [128, 512] tile (e on partitions).
  - out += mixT_norm.T @ Wo: dense K=128 bf16 matmuls accumulating
    over the 4 pairs; DVE evac fp32 -> DMA to HBM. The last qb's Wo
    runs p-major across 6 concurrent PSUM accumulators so only the
    final pair's 6 matmuls + evacs trail the last normalize.

PSUM: sc ring 2x[128,1024] (scores + qb0 projection accumulators +
the tail broadcast) = 4 banks, mix 1x[128,1024] = 2 banks, aux ring
2x[128,512] (projection pj + Wo accumulators) = 2 banks.
"""

import numpy as np

B, S, D = 4, 2048, 1024
H, DK, DV = 16, 64, 64
HC = 8          # heads per core
NP = HC // 2    # head pairs per core
NCORES = 8
NC_CHUNKS = D // 128    # 8 contraction chunks over D
NKC = S // 128          # 16 key chunks
NQB = S // 512          # 4 query blocks
VW = HC * (DV + 1)      # vh storage: 65 cols per head (dv | mask)

_COMPILED = {}


def _build_nc():
    import concourse.tile as tile
    from concourse import bacc, mybir
    from contextlib import ExitStack

    F32 = mybir.dt.float32
    F32R = mybir.dt.float32r
    BF16 = mybir.dt.bfloat16
    EXP = mybir.ActivationFunctionType.Exp
    COPY = mybir.ActivationFunctionType.Copy

    nc = bacc.Bacc("TRN2", target_bir_lowering=False, debug=False,
                   num_devices=NCORES)

    qT = nc.dram_tensor("qT", [D, S], BF16, kind="ExternalInput").ap()
    kT = nc.dram_tensor("kT", [D, S], BF16, kind="ExternalInput").ap()
    vT = nc.dram_tensor("vT", [D, S], BF16, kind="ExternalInput").ap()
    wq = nc.dram_tensor("wq", [D, HC * DK], BF16, kind="ExternalInput").ap()
    wk = nc.dram_tensor("wk", [D, HC * DK], BF16, kind="ExternalInput").ap()
    wv = nc.dram_tensor("wv", [D, HC * DV], BF16, kind="ExternalInput").ap()
    wo = nc.dram_tensor("wo", [HC * DV, D], BF16, kind="ExternalInput").ap()
    maskr = nc.dram_tensor("maskr", [128, NKC], F32, kind="ExternalInput").ap()
    out = nc.dram_tensor("out", [S, D], F32, kind="ExternalOutput").ap()

    with tile.TileContext(nc) as tc:
        with ExitStack() as ctx:
            const_pool = ctx.enter_context(tc.tile_pool(name="const", bufs=1))
            w_pool = ctx.enter_context(tc.tile_pool(name="weights", bufs=1))
            act_pool = ctx.enter_context(tc.tile_pool(name="acts", bufs=1))
            # 5 blocks of 8 stage tiles live at once (k0-k3 + q0); later
            # q blocks wrap onto long-drained slots
            st_pool = ctx.enter_context(
                tc.tile_pool(name="stage", bufs=5 * NC_CHUNKS))
            vt_pool = ctx.enter_context(tc.tile_pool(name="vstage", bufs=2))
            exp_pool = ctx.enter_context(tc.tile_pool(name="exp", bufs=6))
            norm_pool = ctx.enter_context(tc.tile_pool(name="norm", bufs=2 * NP))
            rec_pool = ctx.enter_context(tc.tile_pool(name="rec", bufs=4))
            sums_pool = ctx.enter_context(tc.tile_pool(name="sums", bufs=2))
            sh_pool = ctx.enter_context(tc.tile_pool(name="sh", bufs=4))
            osb_pool = ctx.enter_context(tc.tile_pool(name="outsb", bufs=4))
            sc_pool = ctx.enter_context(
                tc.tile_pool(name="scpsum", bufs=2, space="PSUM"))
            mix_pool = ctx.enter_context(
                tc.tile_pool(name="mxpsum", bufs=1, space="PSUM"))
            aux_pool = ctx.enter_context(
                tc.tile_pool(name="auxpsum", bufs=2, space="PSUM"))
            dram_pool = ctx.enter_context(
                tc.tile_pool(name="dscratch", bufs=4, space="DRAM"))

            mask_sb = const_pool.tile([128, NKC], F32)
            nc.sync.dma_start(mask_sb[:], maskr[:])
            # PE warmup: the PE would otherwise idle ~13us waiting for
            # the first weight/staging DMAs, then pay the 0.65/1.2GHz
            # p-state ramp on the real projections. Dummy matmuls on
            # resident SBUF data bridge the wait at full ramp.
            warm_sb = const_pool.tile([128, 512], BF16)
            nc.vector.memset(warm_sb[:], 0.0)
            warm_ps = aux_pool.tile([128, 512], F32, tag="aux",
                                    name="warmps")
            for i in range(48):
                nc.tensor.matmul(warm_ps[:], lhsT=warm_sb[:, 0:128],
                                 rhs=warm_sb[:],
                                 start=(i == 0), stop=(i == 47))
            warm_out = const_pool.tile([128, 512], F32)
            nc.vector.tensor_copy(warm_out[:], warm_ps[:])
            ones_sb = const_pool.tile([128, HC], BF16)
            nc.vector.memset(ones_sb[:], 1.0)
            # K=1 f32r ones row for the tail's PE sum-broadcast
            # (memset cannot write f32r; bounce through an f32 tile)
            e_ones32 = const_pool.tile([65, 64], F32)
            nc.vector.memset(e_ones32[:], 1.0)
            e_ones = const_pool.tile([65, 64], F32R)
            nc.vector.tensor_copy(e_ones[:], e_ones32[:])

            # DMA priority order: the head's critical path is
            # wk + k staging (first projections), then wq + q staging;
            # wv is needed at attention start and wo only at qb1.
            wq_sb = w_pool.tile([128, NC_CHUNKS * 512], BF16, tag="wq")
            wk_sb = w_pool.tile([128, NC_CHUNKS * 512], BF16, tag="wk")
            wv_sb = w_pool.tile([128, NC_CHUNKS * 512], BF16, tag="wv")
            wo_sb = w_pool.tile([128, NP * 1024], BF16, tag="wo")
            # head critical path: wk then kT-block0, whole chunks (1KB
            # partition lines - smaller splits lose DMA efficiency)
            for c in range(NC_CHUNKS):
                nc.sync.dma_start(wk_sb[:, c * 512:(c + 1) * 512],
                                  wk[c * 128:(c + 1) * 128, :])

            # persistent activations. khT/vh are single tiles (written
            # in slices, region-tracked) so consecutive attention
            # matmuls switch lhsT within one tile - cheaper LDWEIGHTS.
            qhTb = [[act_pool.tile([128, 512], BF16, tag=f"qhT{p}_{b}",
                                   name=f"qhT{p}_{b}") for b in range(NQB)]
                    for p in range(NP)]
            khT = [act_pool.tile([128, S], BF16, tag=f"khT{p}",
                                 name=f"khT{p}") for p in range(NP)]
            khTb = [[khT[p][:, b * 512:(b + 1) * 512] for b in range(NQB)]
                    for p in range(NP)]
            vhs_all = act_pool.tile([128, NKC * VW], BF16, tag="vhall")
            vhs = [vhs_all[:, t * VW:(t + 1) * VW] for t in range(NKC)]

            def stage_block(src, blk, split=False):
                stg = []
                for c in range(NC_CHUNKS):
                    t = st_pool.tile([128, 512], BF16, tag="stage",
                                     name=f"stg{c}")
                    nsp = (4 if c < 2 else 2) if split else 1
                    w = 512 // nsp
                    for h in range(nsp):
                        nc.sync.dma_start(
                            t[:, h * w:(h + 1) * w],
                            src[c * 128:(c + 1) * 128,
                                blk * 512 + h * w:blk * 512 + (h + 1) * w])
                    stg.append(t)
                return stg

            def proj_pair(stg, wsb, dst_tile, p, pool=None, act_evac=False):
                if pool is None:
                    ps = aux_pool.tile([128, 512], F32, tag="aux")
                else:
                    # borrow half an sc-ring tile (qb0 fillers: the sc
                    # ring has slack while attention is PE-bound)
                    pst = pool.tile([128, 1024], F32, tag="sc", name="pjsc")
                    ps = pst[:, 0:512]
                for c in range(NC_CHUNKS):
                    nc.tensor.matmul(
                        ps[:],
                        lhsT=wsb[:, c * 512 + p * 128:
                                 c * 512 + (p + 1) * 128],
                        rhs=stg[c][:],
                        start=(c == 0), stop=(c == NC_CHUNKS - 1))
                if act_evac:
                    # qb0 era: ScalarE is idle; keep DVE off the critical
                    # path of PSUM slot recycling
                    nc.scalar.activation(dst_tile[:], ps[:], COPY)
                else:
                    nc.vector.tensor_copy(dst_tile[:], ps[:])

            # vT is staged in 512-token quarters (large 1KB-line DMAs,
            # double buffered) so v-proj matmuls never wait on staging
            vt_cur = {}

            def stage_vq(vq):
                vt = vt_pool.tile([128, NC_CHUNKS * 512], BF16, tag="vt",
                                  name=f"vq{vq}")
                for c in range(NC_CHUNKS):
                    nc.sync.dma_start(
                        vt[:, c * 512:(c + 1) * 512],
                        vT[c * 128:(c + 1) * 128,
                           vq * 512:(vq + 1) * 512])
                vt_cur[vq] = vt

            def vproj_chunk(t):
                vt = vt_cur[t // 4]
                o = t % 4
                ps = aux_pool.tile([128, 512], F32, tag="aux")
                for c in range(NC_CHUNKS):
                    nc.tensor.matmul(
                        ps[:],
                        lhsT=vt[:, c * 512 + o * 128:
                                c * 512 + (o + 1) * 128],
                        rhs=wv_sb[:, c * 512:(c + 1) * 512],
                        start=(c == 0), stop=(c == NC_CHUNKS - 1))
                dst_dv = vhs[t][:, 0:VW].rearrange(
                    "p (h x) -> p h x", x=DV + 1)[:, :, 0:DV]
                src_dv = ps[:].rearrange("p (h x) -> p h x", x=DV)
                nc.scalar.activation(dst_dv, src_dv, COPY,
                                     scale=mask_sb[:, t:t + 1])
                dst_m = vhs[t][:, 0:VW].rearrange(
                    "p (h x) -> p h x", x=DV + 1)[:, :, DV:DV + 1]
                src_m = ones_sb[:, 0:HC].rearrange("p (h x) -> p h x", x=1)
                nc.vector.tensor_scalar_mul(dst_m, src_m,
                                            mask_sb[:, t:t + 1])

            # Wo for one (qb, tt, dh) triple, issued one matmul at a time
            # (4 accumulating steps + evac) so it spreads as PE filler
            wo_state = {}

            def wo_single(qb, normT, tt, dh, p):
                if p == 0:
                    wo_state[(tt, dh)] = aux_pool.tile(
                        [128, 512], F32, tag="aux", name=f"wps{tt}{dh}")
                wps = wo_state[(tt, dh)]
                nc.tensor.matmul(
                    wps[:],
                    lhsT=normT[p][:, tt * 128:(tt + 1) * 128],
                    rhs=wo_sb[:, p * 1024 + dh * 512:
                              p * 1024 + (dh + 1) * 512],
                    start=(p == 0), stop=(p == NP - 1))
                if p == NP - 1:
                    osb = osb_pool.tile([128, 512], F32, tag="osb")
                    nc.vector.tensor_copy(osb[:], wps[:])
                    nc.sync.dma_start(
                        out[qb * 512 + tt * 128:qb * 512 + (tt + 1) * 128,
                            dh * 512:(dh + 1) * 512], osb[:])

            def wo_piece(qb, normT, tt, dh):
                for p in range(NP):
                    wo_single(qb, normT, tt, dh, p)

            # ---- stage + project k block 0 / q block 0 for pair 0 ----
            kstg = [None] * NQB
            qstg = [None] * NQB
            kstg[0] = stage_block(kT, 0)
            for c in range(NC_CHUNKS):
                nc.sync.dma_start(wq_sb[:, c * 512:(c + 1) * 512],
                                  wq[c * 128:(c + 1) * 128, :])
            qstg[0] = stage_block(qT, 0)
            for c in range(NC_CHUNKS):
                nc.sync.dma_start(wv_sb[:, c * 512:(c + 1) * 512],
                                  wv[c * 128:(c + 1) * 128, :])
            proj_pair(kstg[0], wk_sb, khTb[0][0], 0, act_evac=True)
            proj_pair(qstg[0], wq_sb, qhTb[0][0], 0, act_evac=True)
            stage_vq(0)
            stage_vq(1)
            for p in range(NP):
                nc.sync.dma_start(wo_sb[:, p * 1024:(p + 1) * 1024],
                                  wo[p * 128:(p + 1) * 128, :])

            # filler thunks, one consumed at the top of each attention
            # g-iteration. The order guarantees every tile's write is
            # issued before its first read (pair p's blocks land during
            # pair p-1's loop, with p0's later k blocks interleaved
            # just ahead of the scores that need them).
            def kf(kb, p):
                return lambda: proj_pair(kstg[kb], wk_sb, khTb[p][kb], p,
                                         pool=sc_pool, act_evac=True)

            def qf(qb, p, pool=None):
                return lambda: proj_pair(qstg[qb], wq_sb, qhTb[p][qb], p,
                                         pool=pool,
                                         act_evac=pool is not None)

            # q-projection issued one matmul at a time (steady-state
            # filler granularity; the aux tile is held across 8 slots)
            pj_state = {}

            def qsingle(qb, p, c):
                def f():
                    if c == 0:
                        pj_state[(qb, p)] = aux_pool.tile(
                            [128, 512], F32, tag="aux",
                            name=f"qpj{qb}_{p}")
                    ps = pj_state[(qb, p)]
                    nc.tensor.matmul(
                        ps[:],
                        lhsT=wq_sb[:, c * 512 + p * 128:
                                   c * 512 + (p + 1) * 128],
                        rhs=qstg[qb][c][:],
                        start=(c == 0), stop=(c == NC_CHUNKS - 1))
                    if c == NC_CHUNKS - 1:
                        nc.vector.tensor_copy(qhTb[p][qb][:], ps[:])
                return f

            def qb0_fillers():
                for kb in range(1, NQB):
                    kstg[kb] = stage_block(kT, kb)
                # None: no filler in the very first g iteration - lets
                # the attention pipeline warm up before the sc ring is
                # borrowed for projection accumulators.
                fills = [None, kf(1, 0), kf(0, 1), kf(2, 0), kf(0, 2),
                         kf(3, 0), kf(0, 3), qf(0, 1, sc_pool), kf(1, 1),
                         kf(2, 1), kf(3, 1), qf(0, 2, sc_pool), kf(1, 2),
                         kf(2, 2), kf(3, 2), qf(0, 3, sc_pool), kf(1, 3),
                         kf(2, 3), kf(3, 3)]

                def stage_q1():
                    qstg[1] = stage_block(qT, 1)
                fills.append(stage_q1)
                for p in range(NP):
                    fills.append(qf(1, p, sc_pool))
                return fills

            # completed blocks' Wo matmuls wait in a FIFO consumed across
            # later blocks' filler slots: qb1/qb2 are PE-bound (fillers
            # cost wall-clock 1:1) while qb3 has ACT-bound slack where
            # deferred Wo work is nearly free. Caps are multiples of 4 so
            # a Wo accumulation group never splits across blocks.
            wo_queue = []
            WO_CAP = {1: 24, 2: 28, 3: 10 ** 6}

            def qbn_fillers(qb):
                fills = []
                has_q = qb + 1 < NQB
                if has_q:
                    def stage_qn():
                        qstg[qb + 1] = stage_block(qT, qb + 1)
                    fills.append(stage_qn)
                nwo = min(WO_CAP[qb], len(wo_queue))
                wos = [wo_queue.pop(0) for _ in range(nwo)]
                qi = 0
                # interleave: one 4-matmul Wo group, then one 8-matmul
                # q-projection group (the two aux slots alternate)
                while wos or (has_q and qi < NP):
                    fills += wos[:4]
                    del wos[:4]
                    if has_q and qi < NP:
                        for c in range(NC_CHUNKS):
                            fills.append(qsingle(qb + 1, qi, c))
                        qi += 1
                return fills

            prev_normT = None
            for qb in range(NQB):
                fills = (qb0_fillers() if qb == 0
                         else qbn_fillers(qb))
                fills = iter(fills)

                def filler():
                    f = next(fills, None)
                    if f is not None and callable(f):
                        f()

                normT = []
                for p in range(NP):
                    h0, h1 = 2 * p, 2 * p + 1
                    mix2 = mix_pool.tile([128, 1024], F32, tag="mix")
                    mixP = mix2[:, 0:512]
                    mixR = mix2[:, 512:1024]
                    l0 = slice(h0 * 65, h0 * 65 + 65)
                    l1 = slice(h1 * 65, h1 * 65 + 65)

                    def mix_g(g, exs):
                        ex0, ex1 = exs
                        for s2 in range(2):
                            kc = 2 * g + s2
                            esl = slice(s2 * 512, (s2 + 1) * 512)
                            nc.tensor.matmul(
                                mixP[0:65, :],
                                lhsT=vhs[kc][:, l0], rhs=ex0[:, esl],
                                start=(kc == 0), stop=(kc == NKC - 1))
                        for s2 in range(2):
                            kc = 2 * g + s2
                            esl = slice(s2 * 512, (s2 + 1) * 512)
                            nc.tensor.matmul(
                                mixR[0:65, :],
                                lhsT=vhs[kc][:, l1], rhs=ex1[:, esl],
                                start=(kc == 0), stop=(kc == NKC - 1))

                    # mix for group g is issued one iteration behind its
                    # exp, so the PE never sits adjacent to the ACT
                    # dependency (weights prefetch, no stall).
                    pend = None
                    for g in range(NKC // 2):
                        filler()
                        if not (qb == 0 and p == 0):
                            filler()
                        sc0 = sc_pool.tile([128, 1024], F32, tag="sc")
                        sc1 = sc_pool.tile([128, 1024], F32, tag="sc")
                        for s2 in range(2):
                            kc = 2 * g + s2
                            kb, ko = kc // 4, kc % 4
                            ksl = slice(ko * 128, (ko + 1) * 128)
                            nc.tensor.matmul(
                                sc0[:, s2 * 512:(s2 + 1) * 512],
                                lhsT=khTb[p][kb][0:64, ksl],
                                rhs=qhTb[p][qb][0:64, :],
                                start=True, stop=True)
                            nc.tensor.matmul(
                                sc1[:, s2 * 512:(s2 + 1) * 512],
                                lhsT=khTb[p][kb][64:128, ksl],
                                rhs=qhTb[p][qb][64:128, :],
                                start=True, stop=True)
                        if qb == 0 and p == 0:
                            # project v chunks just-in-time for mix;
                            # prefetch the next vT quarter (ring of 2)
                            if g == 2:
                                stage_vq(2)
                            elif g == 4:
                                stage_vq(3)
                            vproj_chunk(2 * g)
                            vproj_chunk(2 * g + 1)
                        ex0 = exp_pool.tile([128, 1024], BF16, tag="exp")
                        ex1 = exp_pool.tile([128, 1024], BF16, tag="exp")
                        nc.scalar.activation(ex0[:], sc0[:], EXP)
                        nc.scalar.activation(ex1[:], sc1[:], EXP)
                        if pend is not None:
                            mix_g(g - 1, pend)
                        pend = (ex0, ex1)
                    mix_g(NKC // 2 - 1, pend)
                    nt = norm_pool.tile([128, 512], BF16, tag="norm")
                    normT.append(nt)
                    if qb == NQB - 1 and p == NP - 1:
                        # kernel tail: lowest-latency normalize. Broadcast
                        # the sums row with a K=1 f32r matmul (PE is idle
                        # here), multiply straight out of PSUM on DVE.
                        su_r = sums_pool.tile([65, 1024], F32R, tag="sumr",
                                              name="sur")
                        nc.vector.tensor_copy(su_r[64:65, :], mix2[64:65, :])
                        bcp = sc_pool.tile([128, 1024], F32, tag="sc",
                                           name="bcp")
                        nc.tensor.matmul(
                            bcp[0:64, 0:512], lhsT=e_ones[64:65, :],
                            rhs=su_r[64:65, 0:512], start=True, stop=True)
                        nc.tensor.matmul(
                            bcp[0:64, 512:1024], lhsT=e_ones[64:65, :],
                            rhs=su_r[64:65, 512:1024], start=True, stop=True)
                        recb = rec_pool.tile([64, 1024], F32, tag="rec")
                        nc.vector.reciprocal_approx_fast(recb[:],
                                                         bcp[0:64, :])
                        nc.vector.tensor_mul(nt[0:64, :], mix2[0:64, 0:512],
                                             recb[:, 0:512])
                        sh1 = sh_pool.tile([64, 512], BF16, tag="sh1")
                        nc.vector.tensor_mul(sh1[:], mix2[0:64, 512:1024],
                                             recb[:, 512:1024])
                        nc.sync.dma_start(nt[64:128, :], sh1[:])
                    else:
                        # normalize (no PE involvement): evacuate mix rows
                        # 0:65 to SBUF (frees the PSUM tile), bounce the
                        # sums row through DRAM to broadcast it over
                        # partitions, reciprocal on DVE, scale on GpSimd.
                        madd = sums_pool.tile([65, 1024], F32, tag="sums")
                        nc.vector.tensor_copy(madd[:], mix2[0:65, :])
                        dsc = dram_pool.tile([1, 1024], F32, tag="dsc")
                        nc.sync.dma_start(dsc[:], madd[64:65, :])
                        rin = rec_pool.tile([64, 1024], F32, tag="rec")
                        nc.sync.dma_start(
                            rin[:], dsc[0:1, :].to_broadcast((64, 1024)))
                        recb = rec_pool.tile([64, 1024], F32, tag="rec")
                        nc.vector.reciprocal_approx_fast(recb[:], rin[:])
                        nc.gpsimd.tensor_mul(nt[0:64, :], madd[0:64, 0:512],
                                             recb[:, 0:512])
                        sh1 = sh_pool.tile([64, 512], BF16, tag="sh1")
                        nc.gpsimd.tensor_mul(sh1[:], madd[0:64, 512:1024],
                                             recb[:, 512:1024])
                        nc.sync.dma_start(nt[64:128, :], sh1[:])

                # drain any unissued fillers for this qb
                for f in fills:
                    if callable(f):
                        f()
                prev_normT = normT
                if qb < NQB - 1:
                    for tt in range(4):
                        for dh in range(2):
                            for p in range(NP):
                                wo_queue.append(
                                    lambda qb=qb, normT=normT, tt=tt,
                                    dh=dh, p=p: wo_single(
                                        qb, normT, tt, dh, p))

            # final block's Wo: 6 groups held concurrently across the
            # aux + sc PSUM banks, issued p-major so the p0-p2 matmuls
            # execute during the last pair's (ACT-bound) attention; only
            # the p3 matmuls + evacs trail the last normalize.
            combos = [(tt, dh) for tt in range(4) for dh in range(2)]
            wo_tiles = [
                aux_pool.tile([128, 512], F32, tag="aux", name="fwa0"),
                aux_pool.tile([128, 512], F32, tag="aux", name="fwa1"),
            ]
            fwsc0 = sc_pool.tile([128, 1024], F32, tag="sc", name="fwsc0")
            fwsc1 = sc_pool.tile([128, 1024], F32, tag="sc", name="fwsc1")
            wo_tiles += [fwsc0[:, 0:512], fwsc0[:, 512:1024],
                         fwsc1[:, 0:512], fwsc1[:, 512:1024]]
            for p in range(NP):
                for i in range(6):
                    tt, dh = combos[i]
                    nc.tensor.matmul(
                        wo_tiles[i][:],
                        lhsT=prev_normT[p][:, tt * 128:(tt + 1) * 128],
                        rhs=wo_sb[:, p * 1024 + dh * 512:
                                  p * 1024 + (dh + 1) * 512],
                        start=(p == 0), stop=(p == NP - 1))
            for i in range(6):
                tt, dh = combos[i]
                osb = osb_pool.tile([128, 512], F32, tag="osb")
                nc.vector.tensor_copy(osb[:], wo_tiles[i][:])
                nc.sync.dma_start(
                    out[(NQB - 1) * 512 + tt * 128:
                        (NQB - 1) * 512 + (tt + 1) * 128,
                        dh * 512:(dh + 1) * 512], osb[:])
            for tt, dh in combos[6:]:
                wo_piece(NQB - 1, prev_normT, tt, dh)

    nc.compile()
    return nc


def _get_nc():
    if "nc" not in _COMPILED:
        _COMPILED["nc"] = _build_nc()
    return _COMPILED["nc"]


def _shard_inputs(q, k, v, mask, Wq, Wk, Wv, Wo):
    """Build the per-core input maps (host-side layout prep)."""
    import ml_dtypes

    bf16 = ml_dtypes.bfloat16
    in_maps = []
    maskf = np.asarray(mask).astype(np.float32)
    q = np.asarray(q, np.float32)
    k = np.asarray(k, np.float32)
    v = np.asarray(v, np.float32)
    Wq = np.asarray(Wq, np.float32)
    Wk = np.asarray(Wk, np.float32)
    Wv = np.asarray(Wv, np.float32)
    Wo = np.asarray(Wo, np.float32)
    scale = np.float32(1.0 / np.sqrt(DK))
    for c in range(NCORES):
        b, hg = c // 2, c % 2
        hs = hg * HC
        m = {
            "qT": np.ascontiguousarray(q[b].T).astype(bf16),
            "kT": np.ascontiguousarray(k[b].T).astype(bf16),
            "vT": np.ascontiguousarray(v[b].T).astype(bf16),
            # head-major col blocks; fold 1/sqrt(dk) into Wq
            "wq": np.ascontiguousarray(
                Wq[hs:hs + HC].transpose(1, 0, 2).reshape(D, HC * DK) * scale
            ).astype(bf16),
            "wk": np.ascontiguousarray(
                Wk[hs:hs + HC].transpose(1, 0, 2).reshape(D, HC * DK)
            ).astype(bf16),
            "wv": np.ascontiguousarray(
                Wv[hs:hs + HC].transpose(1, 0, 2).reshape(D, HC * DV)
            ).astype(bf16),
            "wo": np.ascontiguousarray(Wo[hs * DV:(hs + HC) * DV]).astype(bf16),
            "maskr": np.ascontiguousarray(
                maskf[b].reshape(NKC, 128).T).astype(np.float32),
        }
        in_maps.append(m)
    return in_maps


def kernel(q, k, v, mask, Wq, Wk, Wv, Wo, _trace=False):
    from concourse.bass_utils import run_bass_kernel_spmd

    nc = _get_nc()
    in_maps = _shard_inputs(q, k, v, mask, Wq, Wk, Wv, Wo)
    res = run_bass_kernel_spmd(nc, in_maps, list(range(NCORES)),
                               trace=_trace)
    out = np.zeros((B, S, D), np.float32)
    for c in range(NCORES):
        out[c // 2] += res.results[c]["out"]
    if _trace:
        _COMPILED["last_result"] = res
    return out


# revision 85
# speedup vs baseline: 1.0021x; 1.0014x over previous
"""Multi-head attention (B=4, S=2048, D=1024, H=16, dk=dv=64) on 8 TRN2 cores.

Sharding: core c = 2*b + hg handles batch b = c//2 and heads
[hg*8, hg*8+8). Each core computes a partial output
(its 8 heads' contribution through Wo); the host adds the two partials
per batch.

Per-core device pipeline (matmul inputs bf16, PSUM accumulation fp32).
The kernel is issue-ordered so the list scheduler keeps the PE gap-free
(p-state at max) and the ACT exp stream (the second-busiest engine)
starts ~14us in and never starves:

  - k-proj block 0 and q-proj(qb0,p0) are issued first; attention
    (qb0,p0) scores begin immediately after.
  - all remaining projection work (k blocks 1-3, q pairs, v chunks) is
    issued as PE filler interleaved into the attention g-loops of qb0,
    so the PE always has ready work while exp(g) -> mix(g) dependencies
    drain. v chunk t is projected just-in-time before mix needs it.
  - for qb>0, the fillers are the previous qb's Wo matmuls and the next
    qb's q projection.
  - scores^T per head pair are K=64 matmuls on partition halves
    (h0: partitions 0:64 -> PE tile (0,0); h1: 64:128 -> tile (64,0)),
    [128 keys, 512 q] fp32 in PSUM, two key chunks per [128,1024] PSUM
    tile so each ScalarE exp instruction covers 2 banks.
  - mix^T + softmax sums in one matmul: lhsT = vh_aug [128 keys, 65]
    (col 64 = mask), rhs = exp chunk half; h0/h1 accumulate into the
    two banks of one [128,1024] PSUM tile over the 16 key chunks.
  - mix for group g is issued one g-iteration behind its exp so the PE
    never sits adjacent to the ACT dependency (weights prefetch).
  - normalize (PE-free): evacuate mix rows 0:65 to SBUF (frees the
    PSUM tile after one DVE op), bounce the sums row (partition 64)
    through a DRAM scratch tile to broadcast it over 64 partitions,
    reciprocal on DVE, scale multiplies on the idle GpSimd. The final
    pair instead uses a K=1 f32r PE-broadcast matmul + DVE multiplies
    straight out of PSUM (lowest latency - it is the kernel tail).
    h1's normalized tile is DMA-shifted to partitions 64-127 so each
    pair's mix^T is one [128, 512] tile (e on partitions).
  - out += mixT_norm.T @ Wo: dense K=128 bf16 matmuls accumulating
    over the 4 pairs; DVE evac fp32 -> DMA to HBM. The last qb's Wo
    runs p-major across 6 concurrent PSUM accumulators so only the
    final pair's 6 matmuls + evacs trail the last normalize.

PSUM: sc ring 2x[128,1024] (scores + qb0 projection accumulators +
the tail broadcast) = 4 banks, mix 1x[128,1024] = 2 banks, aux ring
2x[128,512] (projection pj + Wo accumulators) = 2 banks.
"""

import numpy as np

B, S, D = 4, 2048, 1024
H, DK, DV = 16, 64, 64
HC = 8          # heads per core
NP = HC // 2    # head pairs per core
NCORES = 8
NC_CHUNKS = D // 128    # 8 contraction chunks over D
NKC = S // 128          # 16 key chunks
NQB = S // 512          # 4 query blocks
VW = HC * (DV + 1)      # vh storage: 65 cols per head (dv | mask)

_COMPILED = {}


def _build_nc():
    import concourse.tile as tile
    from concourse import bacc, mybir
    from contextlib import ExitStack

    F32 = mybir.dt.float32
    F32R = mybir.dt.float32r
    BF16 = mybir.dt.bfloat16
    EXP = mybir.ActivationFunctionType.Exp
    COPY = mybir.ActivationFunctionType.Copy

    nc = bacc.Bacc("TRN2", target_bir_lowering=False, debug=False,
                   num_devices=NCORES)

    qT = nc.dram_tensor("qT", [D, S], BF16, kind="ExternalInput").ap()
    kT = nc.dram_tensor("kT", [D, S], BF16, kind="ExternalInput").ap()
    vT = nc.dram_tensor("vT", [D, S], BF16, kind="ExternalInput").ap()
    wq = nc.dram_tensor("wq", [D, HC * DK], BF16, kind="ExternalInput").ap()
    wk = nc.dram_tensor("wk", [D, HC * DK], BF16, kind="ExternalInput").ap()
    wv = nc.dram_tensor("wv", [D, HC * DV], BF16, kind="ExternalInput").ap()
    wo = nc.dram_tensor("wo", [HC * DV, D], BF16, kind="ExternalInput").ap()
    maskr = nc.dram_tensor("maskr", [128, NKC], F32, kind="ExternalInput").ap()
    out = nc.dram_tensor("out", [S, D], F32, kind="ExternalOutput").ap()

    with tile.TileContext(nc) as tc:
        with ExitStack() as ctx:
            const_pool = ctx.enter_context(tc.tile_pool(name="const", bufs=1))
            w_pool = ctx.enter_context(tc.tile_pool(name="weights", bufs=1))
            act_pool = ctx.enter_context(tc.tile_pool(name="acts", bufs=1))
            # 5 blocks of 8 stage tiles live at once (k0-k3 + q0); later
            # q blocks wrap onto long-drained slots
            st_pool = ctx.enter_context(
                tc.tile_pool(name="stage", bufs=5 * NC_CHUNKS))
            vt_pool = ctx.enter_context(tc.tile_pool(name="vstage", bufs=2))
            exp_pool = ctx.enter_context(tc.tile_pool(name="exp", bufs=6))
            norm_pool = ctx.enter_context(tc.tile_pool(name="norm", bufs=2 * NP))
            rec_pool = ctx.enter_context(tc.tile_pool(name="rec", bufs=4))
            sums_pool = ctx.enter_context(tc.tile_pool(name="sums", bufs=2))
            sh_pool = ctx.enter_context(tc.tile_pool(name="sh", bufs=4))
            osb_pool = ctx.enter_context(tc.tile_pool(name="outsb", bufs=4))
            sc_pool = ctx.enter_context(
                tc.tile_pool(name="scpsum", bufs=2, space="PSUM"))
            mix_pool = ctx.enter_context(
                tc.tile_pool(name="mxpsum", bufs=1, space="PSUM"))
            aux_pool = ctx.enter_context(
                tc.tile_pool(name="auxpsum", bufs=2, space="PSUM"))
            dram_pool = ctx.enter_context(
                tc.tile_pool(name="dscratch", bufs=4, space="DRAM"))

            mask_sb = const_pool.tile([128, NKC], F32)
            nc.sync.dma_start(mask_sb[:], maskr[:])
            # PE warmup: the PE would otherwise idle ~13us waiting for
            # the first weight/staging DMAs, then pay the 0.65/1.2GHz
            # p-state ramp on the real projections. Dummy matmuls on
            # resident SBUF data bridge the wait at full ramp.
            warm_sb = const_pool.tile([128, 512], BF16)
            nc.vector.memset(warm_sb[:], 0.0)
            warm_ps = aux_pool.tile([128, 512], F32, tag="aux",
                                    name="warmps")
            for i in range(48):
                nc.tensor.matmul(warm_ps[:], lhsT=warm_sb[:, 0:128],
                                 rhs=warm_sb[:],
                                 start=(i == 0), stop=(i == 47))
            warm_out = const_pool.tile([128, 512], F32)
            nc.vector.tensor_copy(warm_out[:], warm_ps[:])
            ones_sb = const_pool.tile([128, HC], BF16)
            nc.vector.memset(ones_sb[:], 1.0)
            # K=1 f32r ones row for the tail's PE sum-broadcast
            # (memset cannot write f32r; bounce through an f32 tile)
            e_ones32 = const_pool.tile([65, 64], F32)
            nc.vector.memset(e_ones32[:], 1.0)
            e_ones = const_pool.tile([65, 64], F32R)
            nc.vector.tensor_copy(e_ones[:], e_ones32[:])

            # DMA priority order: the head's critical path is
            # wk + k staging (first projections), then wq + q staging;
            # wv is needed at attention start and wo only at qb1.
            wq_sb = w_pool.tile([128, NC_CHUNKS * 512], BF16, tag="wq")
            wk_sb = w_pool.tile([128, NC_CHUNKS * 512], BF16, tag="wk")
            wv_sb = w_pool.tile([128, NC_CHUNKS * 512], BF16, tag="wv")
            wo_sb = w_pool.tile([128, NP * 1024], BF16, tag="wo")
            # head critical path: wk then kT-block0, whole chunks (1KB
            # partition lines - smaller splits lose DMA efficiency)
            for c in range(NC_CHUNKS):
                nc.sync.dma_start(wk_sb[:, c * 512:(c + 1) * 512],
                                  wk[c * 128:(c + 1) * 128, :])

            # persistent activations. khT/vh are single tiles (written
            # in slices, region-tracked) so consecutive attention
            # matmuls switch lhsT within one tile - cheaper LDWEIGHTS.
            qhTb = [[act_pool.tile([128, 512], BF16, tag=f"qhT{p}_{b}",
                                   name=f"qhT{p}_{b}") for b in range(NQB)]
                    for p in range(NP)]
            khT = [act_pool.tile([128, S], BF16, tag=f"khT{p}",
                                 name=f"khT{p}") for p in range(NP)]
            khTb = [[khT[p][:, b * 512:(b + 1) * 512] for b in range(NQB)]
                    for p in range(NP)]
            vhs_all = act_pool.tile([128, NKC * VW], BF16, tag="vhall")
            vhs = [vhs_all[:, t * VW:(t + 1) * VW] for t in range(NKC)]

            def stage_block(src, blk, split=False):
                stg = []
                for c in range(NC_CHUNKS):
                    t = st_pool.tile([128, 512], BF16, tag="stage",
                                     name=f"stg{c}")
                    nsp = (4 if c < 2 else 2) if split else 1
                    w = 512 // nsp
                    for h in range(nsp):
                        nc.sync.dma_start(
                            t[:, h * w:(h + 1) * w],
                            src[c * 128:(c + 1) * 128,
                                blk * 512 + h * w:blk * 512 + (h + 1) * w])
                    stg.append(t)
                return stg

            def proj_pair(stg, wsb, dst_tile, p, pool=None, act_evac=False):
                if pool is None:
                    ps = aux_pool.tile([128, 512], F32, tag="aux")
                else:
                    # borrow half an sc-ring tile (qb0 fillers: the sc
                    # ring has slack while attention is PE-bound)
                    pst = pool.tile([128, 1024], F32, tag="sc", name="pjsc")
                    ps = pst[:, 0:512]
                for c in range(NC_CHUNKS):
                    nc.tensor.matmul(
                        ps[:],
                        lhsT=wsb[:, c * 512 + p * 128:
                                 c * 512 + (p + 1) * 128],
                        rhs=stg[c][:],
                        start=(c == 0), stop=(c == NC_CHUNKS - 1))
                if act_evac:
                    # qb0 era: ScalarE is idle; keep DVE off the critical
                    # path of PSUM slot recycling
                    nc.scalar.activation(dst_tile[:], ps[:], COPY)
                else:
                    nc.vector.tensor_copy(dst_tile[:], ps[:])

            # vT is staged in 512-token quarters (large 1KB-line DMAs,
            # double buffered) so v-proj matmuls never wait on staging
            vt_cur = {}

            def stage_vq(vq):
                vt = vt_pool.tile([128, NC_CHUNKS * 512], BF16, tag="vt",
                                  name=f"vq{vq}")
                for c in range(NC_CHUNKS):
                    nc.sync.dma_start(
                        vt[:, c * 512:(c + 1) * 512],
                        vT[c * 128:(c + 1) * 128,
                           vq * 512:(vq + 1) * 512])
                vt_cur[vq] = vt

            def vproj_chunk(t):
                vt = vt_cur[t // 4]
                o = t % 4
                ps = aux_pool.tile([128, 512], F32, tag="aux")
                for c in range(NC_CHUNKS):
                    nc.tensor.matmul(
                        ps[:],
                        lhsT=vt[:, c * 512 + o * 128:
                                c * 512 + (o + 1) * 128],
                        rhs=wv_sb[:, c * 512:(c + 1) * 512],
                        start=(c == 0), stop=(c == NC_CHUNKS - 1))
                dst_dv = vhs[t][:, 0:VW].rearrange(
                    "p (h x) -> p h x", x=DV + 1)[:, :, 0:DV]
                src_dv = ps[:].rearrange("p (h x) -> p h x", x=DV)
                nc.scalar.activation(dst_dv, src_dv, COPY,
                                     scale=mask_sb[:, t:t + 1])
                dst_m = vhs[t][:, 0:VW].rearrange(
                    "p (h x) -> p h x", x=DV + 1)[:, :, DV:DV + 1]
                src_m = ones_sb[:, 0:HC].rearrange("p (h x) -> p h x", x=1)
                nc.vector.tensor_scalar_mul(dst_m, src_m,
                                            mask_sb[:, t:t + 1])

            # Wo for one (qb, tt, dh) triple, issued one matmul at a time
            # (4 accumulating steps + evac) so it spreads as PE filler
            wo_state = {}

            def wo_single(qb, normT, tt, dh, p):
                if p == 0:
                    wo_state[(tt, dh)] = aux_pool.tile(
                        [128, 512], F32, tag="aux", name=f"wps{tt}{dh}")
                wps = wo_state[(tt, dh)]
                nc.tensor.matmul(
                    wps[:],
                    lhsT=normT[p][:, tt * 128:(tt + 1) * 128],
                    rhs=wo_sb[:, p * 1024 + dh * 512:
                              p * 1024 + (dh + 1) * 512],
                    start=(p == 0), stop=(p == NP - 1))
                if p == NP - 1:
                    osb = osb_pool.tile([128, 512], F32, tag="osb")
                    nc.vector.tensor_copy(osb[:], wps[:])
                    nc.sync.dma_start(
                        out[qb * 512 + tt * 128:qb * 512 + (tt + 1) * 128,
                            dh * 512:(dh + 1) * 512], osb[:])

            def wo_piece(qb, normT, tt, dh):
                for p in range(NP):
                    wo_single(qb, normT, tt, dh, p)

            # ---- stage + project k block 0 / q block 0 for pair 0 ----
            kstg = [None] * NQB
            qstg = [None] * NQB
            kstg[0] = stage_block(kT, 0)
            for c in range(NC_CHUNKS):
                nc.sync.dma_start(wq_sb[:, c * 512:(c + 1) * 512],
                                  wq[c * 128:(c + 1) * 128, :])
            qstg[0] = stage_block(qT, 0)
            for c in range(NC_CHUNKS):
                nc.sync.dma_start(wv_sb[:, c * 512:(c + 1) * 512],
                                  wv[c * 128:(c + 1) * 128, :])
            proj_pair(kstg[0], wk_sb, khTb[0][0], 0, act_evac=True)
            proj_pair(qstg[0], wq_sb, qhTb[0][0], 0, act_evac=True)
            stage_vq(0)
            stage_vq(1)
            for p in range(NP):
                nc.sync.dma_start(wo_sb[:, p * 1024:(p + 1) * 1024],
                                  wo[p * 128:(p + 1) * 128, :])

            # filler thunks, one consumed at the top of each attention
            # g-iteration. The order guarantees every tile's write is
            # issued before its first read (pair p's blocks land during
            # pair p-1's loop, with p0's later k blocks interleaved
            # just ahead of the scores that need them).
            def kf(kb, p):
                return lambda: proj_pair(kstg[kb], wk_sb, khTb[p][kb], p,
                                         pool=sc_pool, act_evac=True)

            def qf(qb, p, pool=None):
                return lambda: proj_pair(qstg[qb], wq_sb, qhTb[p][qb], p,
                                         pool=pool,
                                         act_evac=pool is not None)

            # q-projection issued one matmul at a time (steady-state
            # filler granularity; the aux tile is held across 8 slots)
            pj_state = {}

            def qsingle(qb, p, c):
                def f():
                    if c == 0:
                        pj_state[(qb, p)] = aux_pool.tile(
                            [128, 512], F32, tag="aux",
                            name=f"qpj{qb}_{p}")
                    ps = pj_state[(qb, p)]
                    nc.tensor.matmul(
                        ps[:],
                        lhsT=wq_sb[:, c * 512 + p * 128:
                                   c * 512 + (p + 1) * 128],
                        rhs=qstg[qb][c][:],
                        start=(c == 0), stop=(c == NC_CHUNKS - 1))
                    if c == NC_CHUNKS - 1:
                        nc.vector.tensor_copy(qhTb[p][qb][:], ps[:])
                return f

            def qb0_fillers():
                for kb in range(1, NQB):
                    kstg[kb] = stage_block(kT, kb)
                # None: no filler in the very first g iteration - lets
                # the attention pipeline warm up before the sc ring is
                # borrowed for projection accumulators.
                fills = [None, kf(1, 0), kf(0, 1), kf(2, 0), kf(0, 2),
                         kf(3, 0), kf(0, 3), qf(0, 1, sc_pool), kf(1, 1),
                         kf(2, 1), kf(3, 1), qf(0, 2, sc_pool), kf(1, 2),
                         kf(2, 2), kf(3, 2), qf(0, 3, sc_pool), kf(1, 3),
                         kf(2, 3), kf(3, 3)]

                def stage_q1():
                    qstg[1] = stage_block(qT, 1)
                fills.append(stage_q1)
                for p in range(NP):
                    fills.append(qf(1, p, sc_pool))
                return fills

            # completed blocks' Wo matmuls wait in a FIFO consumed across
            # later blocks' filler slots: qb1/qb2 are PE-bound (fillers
            # cost wall-clock 1:1) while qb3 has ACT-bound slack where
            # deferred Wo work is nearly free. Caps are multiples of 4 so
            # a Wo accumulation group never splits across blocks.
            wo_queue = []
            WO_CAP = {1: 20, 2: 24, 3: 10 ** 6}

            def qbn_fillers(qb):
                fills = []
                has_q = qb + 1 < NQB
                if has_q:
                    def stage_qn():
                        qstg[qb + 1] = stage_block(qT, qb + 1)
                    fills.append(stage_qn)
                nwo = min(WO_CAP[qb], len(wo_queue))
                wos = [wo_queue.pop(0) for _ in range(nwo)]
                qi = 0
                # interleave: one 4-matmul Wo group, then one 8-matmul
                # q-projection group (the two aux slots alternate)
                while wos or (has_q and qi < NP):
                    fills += wos[:4]
                    del wos[:4]
                    if has_q and qi < NP:
                        for c in range(NC_CHUNKS):
                            fills.append(qsingle(qb + 1, qi, c))
                        qi += 1
                return fills

            prev_normT = None
            for qb in range(NQB):
                fills = (qb0_fillers() if qb == 0
                         else qbn_fillers(qb))
                fills = iter(fills)

                def filler():
                    f = next(fills, None)
                    if f is not None and callable(f):
                        f()

                normT = []
                for p in range(NP):
                    h0, h1 = 2 * p, 2 * p + 1
                    mix2 = mix_pool.tile([128, 1024], F32, tag="mix")
                    mixP = mix2[:, 0:512]
                    mixR = mix2[:, 512:1024]
                    l0 = slice(h0 * 65, h0 * 65 + 65)
                    l1 = slice(h1 * 65, h1 * 65 + 65)

                    def mix_g(g, exs):
                        ex0, ex1 = exs
                        for s2 in range(2):
                            kc = 2 * g + s2
                            esl = slice(s2 * 512, (s2 + 1) * 512)
                            nc.tensor.matmul(
                                mixP[0:65, :],
                                lhsT=vhs[kc][:, l0], rhs=ex0[:, esl],
                                start=(kc == 0), stop=(kc == NKC - 1))
                        for s2 in range(2):
                            kc = 2 * g + s2
                            esl = slice(s2 * 512, (s2 + 1) * 512)
                            nc.tensor.matmul(
                                mixR[0:65, :],
                                lhsT=vhs[kc][:, l1], rhs=ex1[:, esl],
                                start=(kc == 0), stop=(kc == NKC - 1))

                    # mix for group g is issued one iteration behind its
                    # exp, so the PE never sits adjacent to the ACT
                    # dependency (weights prefetch, no stall).
                    pend = None
                    for g in range(NKC // 2):
                        filler()
                        if not (qb == 0 and p == 0):
                            filler()
                        sc0 = sc_pool.tile([128, 1024], F32, tag="sc")
                        sc1 = sc_pool.tile([128, 1024], F32, tag="sc")
                        for s2 in range(2):
                            kc = 2 * g + s2
                            kb, ko = kc // 4, kc % 4
                            ksl = slice(ko * 128, (ko + 1) * 128)
                            nc.tensor.matmul(
                                sc0[:, s2 * 512:(s2 + 1) * 512],
                                lhsT=khTb[p][kb][0:64, ksl],
                                rhs=qhTb[p][qb][0:64, :],
                                start=True, stop=True)
                            nc.tensor.matmul(
                                sc1[:, s2 * 512:(s2 + 1) * 512],
                                lhsT=khTb[p][kb][64:128, ksl],
                                rhs=qhTb[p][qb][64:128, :],
                                start=True, stop=True)
                        if qb == 0 and p == 0:
                            # project v chunks just-in-time for mix;
                            # prefetch the next vT quarter (ring of 2)
                            if g == 2:
                                stage_vq(2)
                            elif g == 4:
                                stage_vq(3)
                            vproj_chunk(2 * g)
                            vproj_chunk(2 * g + 1)
                        ex0 = exp_pool.tile([128, 1024], BF16, tag="exp")
                        ex1 = exp_pool.tile([128, 1024], BF16, tag="exp")
                        nc.scalar.activation(ex0[:], sc0[:], EXP)
                        nc.scalar.activation(ex1[:], sc1[:], EXP)
                        if pend is not None:
                            mix_g(g - 1, pend)
                        pend = (ex0, ex1)
                    mix_g(NKC // 2 - 1, pend)
                    nt = norm_pool.tile([128, 512], BF16, tag="norm")
                    normT.append(nt)
                    if qb == NQB - 1 and p == NP - 1:
                        # kernel tail: lowest-latency normalize. Broadcast
                        # the sums row with a K=1 f32r matmul (PE is idle
                        # here), multiply straight out of PSUM on DVE.
                        su_r = sums_pool.tile([65, 1024], F32R, tag="sumr",
                                              name="sur")
                        nc.vector.tensor_copy(su_r[64:65, :], mix2[64:65, :])
                        bcp = sc_pool.tile([128, 1024], F32, tag="sc",
                                           name="bcp")
                        nc.tensor.matmul(
                            bcp[0:64, 0:512], lhsT=e_ones[64:65, :],
                            rhs=su_r[64:65, 0:512], start=True, stop=True)
                        nc.tensor.matmul(
                            bcp[0:64, 512:1024], lhsT=e_ones[64:65, :],
                            rhs=su_r[64:65, 512:1024], start=True, stop=True)
                        recb = rec_pool.tile([64, 1024], F32, tag="rec")
                        nc.vector.reciprocal_approx_fast(recb[:],
                                                         bcp[0:64, :])
                        nc.vector.tensor_mul(nt[0:64, :], mix2[0:64, 0:512],
                                             recb[:, 0:512])
                        sh1 = sh_pool.tile([64, 512], BF16, tag="sh1")
                        nc.vector.tensor_mul(sh1[:], mix2[0:64, 512:1024],
                                             recb[:, 512:1024])
                        nc.sync.dma_start(nt[64:128, :], sh1[:])
                    else:
                        # normalize (no PE involvement): evacuate mix rows
                        # 0:65 to SBUF (frees the PSUM tile), bounce the
                        # sums row through DRAM to broadcast it over
                        # partitions, reciprocal on DVE, scale on GpSimd.
                        madd = sums_pool.tile([65, 1024], F32, tag="sums")
                        nc.vector.tensor_copy(madd[:], mix2[0:65, :])
                        dsc = dram_pool.tile([1, 1024], F32, tag="dsc")
                        nc.sync.dma_start(dsc[:], madd[64:65, :])
                        rin = rec_pool.tile([64, 1024], F32, tag="rec")
                        nc.sync.dma_start(
                            rin[:], dsc[0:1, :].to_broadcast((64, 1024)))
                        recb = rec_pool.tile([64, 1024], F32, tag="rec")
                        nc.vector.reciprocal_approx_fast(recb[:], rin[:])
                        nc.gpsimd.tensor_mul(nt[0:64, :], madd[0:64, 0:512],
                                             recb[:, 0:512])
                        sh1 = sh_pool.tile([64, 512], BF16, tag="sh1")
                        nc.gpsimd.tensor_mul(sh1[:], madd[0:64, 512:1024],
                                             recb[:, 512:1024])
                        nc.sync.dma_start(nt[64:128, :], sh1[:])

                # drain any unissued fillers for this qb
                for f in fills:
                    if callable(f):
                        f()
                prev_normT = normT
                if qb < NQB - 1:
                    for tt in range(4):
                        for dh in range(2):
                            for p in range(NP):
                                wo_queue.append(
                                    lambda qb=qb, normT=normT, tt=tt,
                                    dh=dh, p=p: wo_single(
                                        qb, normT, tt, dh, p))

            # final block's Wo: 6 groups held concurrently across the
            # aux + sc PSUM banks, issued p-major so the p0-p2 matmuls
            # execute during the last pair's (ACT-bound) attention; only
            # the p3 matmuls + evacs trail the last normalize.
            combos = [(tt, dh) for tt in range(4) for dh in range(2)]
            wo_tiles = [
                aux_pool.tile([128, 512], F32, tag="aux", name="fwa0"),
                aux_pool.tile([128, 512], F32, tag="aux", name="fwa1"),
            ]
            fwsc0 = sc_pool.tile([128, 1024], F32, tag="sc", name="fwsc0")
            fwsc1 = sc_pool.tile([128, 1024], F32, tag="sc", name="fwsc1")
            wo_tiles += [fwsc0[:, 0:512], fwsc0[:, 512:1024],
                         fwsc1[:, 0:512], fwsc1[:, 512:1024]]
            for p in range(NP):
                for i in range(6):
                    tt, dh = combos[i]
                    nc.tensor.matmul(
                        wo_tiles[i][:],
                        lhsT=prev_normT[p][:, tt * 128:(tt + 1) * 128],
                        rhs=wo_sb[:, p * 1024 + dh * 512:
                                  p * 1024 + (dh + 1) * 512],
                        start=(p == 0), stop=(p == NP - 1))
            for i in range(6):
                tt, dh = combos[i]
                osb = osb_pool.tile([128, 512], F32, tag="osb")
                nc.vector.tensor_copy(osb[:], wo_tiles[i][:])
                nc.sync.dma_start(
                    out[(NQB - 1) * 512 + tt * 128:
                        (NQB - 1) * 512 + (tt + 1) * 128,
                        dh * 512:(dh + 1) * 512], osb[:])
            for tt, dh in combos[6:]:
                wo_piece(NQB - 1, prev_normT, tt, dh)

    nc.compile()
    return nc


def _get_nc():
    if "nc" not in _COMPILED:
        _COMPILED["nc"] = _build_nc()
    return _COMPILED["nc"]


def _shard_inputs(q, k, v, mask, Wq, Wk, Wv, Wo):
    """Build the per-core input maps (host-side layout prep)."""
    import ml_dtypes

    bf16 = ml_dtypes.bfloat16
    in_maps = []
    maskf = np.asarray(mask).astype(np.float32)
    q = np.asarray(q, np.float32)
    k = np.asarray(k, np.float32)
    v = np.asarray(v, np.float32)
    Wq = np.asarray(Wq, np.float32)
    Wk = np.asarray(Wk, np.float32)
    Wv = np.asarray(Wv, np.float32)
    Wo = np.asarray(Wo, np.float32)
    scale = np.float32(1.0 / np.sqrt(DK))
    for c in range(NCORES):
        b, hg = c // 2, c % 2
        hs = hg * HC
        m = {
            "qT": np.ascontiguousarray(q[b].T).astype(bf16),
            "kT": np.ascontiguousarray(k[b].T).astype(bf16),
            "vT": np.ascontiguousarray(v[b].T).astype(bf16),
            # head-major col blocks; fold 1/sqrt(dk) into Wq
            "wq": np.ascontiguousarray(
                Wq[hs:hs + HC].transpose(1, 0, 2).reshape(D, HC * DK) * scale
            ).astype(bf16),
            "wk": np.ascontiguousarray(
                Wk[hs:hs + HC].transpose(1, 0, 2).reshape(D, HC * DK)
            ).astype(bf16),
            "wv": np.ascontiguousarray(
                Wv[hs:hs + HC].transpose(1, 0, 2).reshape(D, HC * DV)
            ).astype(bf16),
            "wo": np.ascontiguousarray(Wo[hs * DV:(hs + HC) * DV]).astype(bf16),
            "maskr": np.ascontiguousarray(
                maskf[b].reshape(NKC, 128).T).astype(np.float32),
        }
        in_maps.append(m)
    return in_maps


def kernel(q, k, v, mask, Wq, Wk, Wv, Wo, _trace=False):
    from concourse.bass_utils import run_bass_kernel_spmd

    nc = _get_nc()
    in_maps = _shard_inputs(q, k, v, mask, Wq, Wk, Wv, Wo)
    res = run_bass_kernel_spmd(nc, in_maps, list(range(NCORES)),
                               trace=_trace)
    out = np.zeros((B, S, D), np.float32)
    for c in range(NCORES):
        out[c // 2] += res.results[c]["out"]
    if _trace:
        _COMPILED["last_result"] = res
    return out
